# revision 7
# baseline (speedup 1.0000x reference)
"""Bass/TRN2 kernel for nn_BMM_S8T_S8N_S8T:
    out[b,m,n] = sat_i8(round(alpha * sum_k a[b,m,k] * b[b,n,k]))
with a: (32, 2048, 64) int8, b: (32, 2048, 64) int8, alpha: f32 scalar.

Sharding: batch dim 32 -> 8 cores x 4 batches (pure data parallel, no
cross-core communication).

Per-core design notes:
  - int8 matmul is not supported by the PE; bf16 x bf16 -> f32 PSUM is exact
    for int8 operands (products < 2^16, sums of 64 < 2^24), so inputs are
    converted to bf16 (and pre-transposed to [K, seq] layout) on host.
  - The 4 local batches are stacked in pairs along SBUF partitions:
    partitions 0-63 hold batch 2p's K=64, partitions 64-127 batch 2p+1's.
    Row-tiled matmuls (row groups 0 and 2) run the two batches' K=64
    contractions on the 128x128 PE array.
  - Requant drain (PSUM f32 -> SBUF int8, scale + round-half-even + saturate)
    is the hard floor: only VectorE (0.96 GHz, 1 elem/lane/cyc from f32 PSUM)
    and ScalarE (1.2 GHz, 1 elem/lane/cyc) can read PSUM (16-bit PSUM matmul
    output, which would enable 2x reads, is TRN3+ only). Per-instruction
    overhead is ~79ns (DVE) / ~182ns (ACT), so each m-tile pair's 4096
    output columns are drained as three bank-aligned stripes (1536, 1536,
    1024) instead of four 1024s: PSUM tags s3 ([128,1536] x2, banks 0-5) and
    s2 ([128,1024] x1, banks 6-7) fill all 8 banks, and pool rotation gives
    2 stripes draining + 1 filling with no fill-after-drain serialization.
  - The per-(pair,tile) output tile o01 is [128, 4096] int8 with column
    order [b0 n0:1024 | b1 n0:1024 | b0 n1024:2048 | b1 n1024:2048], so
    (a) the first stripe [0:1536] depends only on the first two b pieces
    (q0, q1), letting drains start gapless as soon as data arrives, and
    (b) each batch's full 2048-column DRAM row is one DMA with 2KB lines
    via a 2-chunk strided SBUF access pattern.
  - Startup is input-supply-bound: drains can't start before enough b/lhs
    data has streamed (q0 ~9.9us after the fixed ~7us framework preamble +
    trigger costs). wa1 is triggered before q1 so m-tiles 1-2 become
    drainable early, and the first three m-tiles' stripes are drained at
    bank granularity ordered by piece arrival (q0-gated banks first), which
    keeps both drain engines gapless from ~10.3us.
  - The 16 DMA engines serve each queue's descriptors in trigger order, so
    per-core input is host-repacked into ONE dram tensor `w`, pieces
    triggered on the sync HWDGE in order of first use (f0 on the scalar
    HWDGE in parallel). Output DMAs ride the sync HWDGE; the final m-tile
    is drained in pinned sub-stripes with its four 128KB quarters fired
    from sync / gpsimd / scalar so the exit tail is a single transfer.
"""

import numpy as np
import ml_dtypes

B, M, N, K = 32, 2048, 2048, 64
NCORES = 8
BPC = B // NCORES          # batches per core (4)
MT = M // 128              # m-tiles per batch (16)
WCOLS = 4 * M              # packed input tensor: 8192 bf16 columns

# o01 chunk map: chunk ci covers o01 cols [512ci, 512ci+512) = (batch, n0)
CHUNKS = [(0, 0), (0, 512), (1, 0), (1, 512), (0, 1024), (0, 1536), (1, 1024), (1, 1536)]

_CACHE = {}


def _build(alpha: float):
    import concourse.bacc as bacc
    import concourse.mybir as mybir
    from concourse.tile import TileContext

    bf16 = mybir.dt.bfloat16
    f32 = mybir.dt.float32
    i8 = mybir.dt.int8

    nc = bacc.Bacc("TRN2")
    w = nc.dram_tensor("w", (128, WCOLS), bf16, kind="ExternalInput")
    out = nc.dram_tensor("out", (BPC, M, N), i8, kind="ExternalOutput")

    # drain-engine load balancing: ns per drain instruction by free-dim
    # (hardware-measured at FD=1024, linearly scaled elsewhere:
    #  DVE ~79ns + FD/0.96GHz, ACT ~182ns + FD/1.2GHz)
    DVE = {512: 612.0, 1024: 1146.0, 1536: 1679.0}
    ACT = {512: 609.0, 1024: 1035.0, 1536: 1462.0}
    WARMUP_MM = 4  # filler matmuls while the first input pieces stream in

    # scratch operand for warm-up matmuls: deliberately never written (the
    # values don't matter and the scratch PSUM bank is never read); a raw
    # (non-pool) tensor so Tile's release tracking doesn't object.
    wz = nc.alloc_sbuf_tensor("wz", [128, 512], bf16)

    with TileContext(nc) as tc:
        with (
            tc.tile_pool(name="inp", bufs=1) as inp_pool,
            tc.tile_pool(name="ps", bufs=2, space="PSUM") as psum_pool,
            tc.tile_pool(name="outp", bufs=7) as out_pool,
        ):
            # Input pieces (w column ranges unchanged from the packing in
            # prep_in_maps); trigger order = per-queue completion order.
            #   f0:  a0 cols 0:128      (scalar HWDGE)  -> m-tile 0 lhs
            #   q0:  b0 cols 0:512, q1: b0 cols 512:1024 -> n 0:1024 rhs
            #   wa1: a0 cols 128:384    (triggered before q1: unlocks
            #        m-tiles 1-2 while q1/bh1 still stream)
            #   bh1: b0 cols 1024:2048  -> n 1024:2048 rhs
            #   wa2: a0 cols 384:1024   -> m-tiles 3-7 lhs
            #   wa3: a0 cols 1024:2048  -> m-tiles 8-15 lhs
            #   tw3: b1 cols 0:2048, tw4: a1 cols 0:2048 -> batch pair 1
            f0 = inp_pool.tile([128, 128], bf16, tag="f0")
            q0 = inp_pool.tile([128, 512], bf16, tag="q0")
            q1 = inp_pool.tile([128, 512], bf16, tag="q1")
            wa1 = inp_pool.tile([128, 256], bf16, tag="wa1")
            bh1 = inp_pool.tile([128, 1024], bf16, tag="bh1")
            wa2 = inp_pool.tile([128, 640], bf16, tag="wa2")
            wa3 = inp_pool.tile([128, 1024], bf16, tag="wa3")
            tw3 = inp_pool.tile([128, 2048], bf16, tag="tw3")
            tw4 = inp_pool.tile([128, 2048], bf16, tag="tw4")
            nc.scalar.dma_start(out=f0[:, :], in_=w[:, 0:128])
            nc.sync.dma_start(out=q0[:, :], in_=w[:, 128:640])
            nc.sync.dma_start(out=wa1[:, :], in_=w[:, 1152:1408])
            nc.sync.dma_start(out=q1[:, :], in_=w[:, 640:1152])
            nc.sync.dma_start(out=bh1[:, :], in_=w[:, 1408:2432])
            nc.sync.dma_start(out=wa2[:, :], in_=w[:, 2432:3072])
            nc.sync.dma_start(out=wa3[:, :], in_=w[:, 3072:4096])
            nc.sync.dma_start(out=tw3[:, :], in_=w[:, 4096:6144])
            nc.sync.dma_start(out=tw4[:, :], in_=w[:, 6144:8192])

            # warm-up matmuls: keep the PE active (HAM clock ramp) while the
            # first input piece lands
            wps = psum_pool.tile([128, 1024], f32, tag="s2", bufs=1, name="wps")
            for _ in range(WARMUP_MM):
                nc.tensor.matmul(
                    wps[:, 0:512], wz[:, 0:128], wz[:, 0:512], start=True, stop=True
                )

            def lhs_ap(p, t, b):
                rows = slice(0, 64) if b == 0 else slice(64, 128)
                if p == 1:
                    return tw4[rows, 128 * t : 128 * (t + 1)]
                if t == 0:
                    return f0[rows, :]
                if t < 3:
                    return wa1[rows, 128 * (t - 1) : 128 * t]
                if t < 8:
                    return wa2[rows, 128 * (t - 3) : 128 * (t - 2)]
                return wa3[rows, 128 * (t - 8) : 128 * (t - 7)]

            def rhs_ap(p, n0, b):
                rows = slice(0, 64) if b == 0 else slice(64, 128)
                if p == 1:
                    return tw3[rows, n0 : n0 + 512]
                if n0 < 512:
                    return q0[rows, :]
                if n0 < 1024:
                    return q1[rows, :]
                return bh1[rows, n0 - 1024 : n0 - 512]

            dve_t = act_t = 0.0

            def drain(o01, cols, ps_ap, fd, force=None):
                nonlocal dve_t, act_t
                use_dve = dve_t + DVE[fd] <= act_t + ACT[fd]
                if force is not None:
                    use_dve = force == "dve"
                if use_dve:
                    nc.vector.tensor_scalar_mul(o01[:, cols], ps_ap, alpha)
                    dve_t += DVE[fd]
                else:
                    nc.scalar.activation(
                        o01[:, cols],
                        ps_ap,
                        mybir.ActivationFunctionType.Copy,
                        scale=alpha,
                    )
                    act_t += ACT[fd]

            def mm(ps, bank, p, t, ci):
                b, n0 = CHUNKS[ci]
                nc.tensor.matmul(
                    ps[:, 512 * bank : 512 * (bank + 1)],
                    lhs_ap(p, t, b),
                    rhs_ap(p, n0, b),
                    start=True,
                    stop=True,
                )

            def tile_dmas(o01, p, t, engines=("sync", "sync")):
                # per-batch paired-chunk DMA: o01 cols {b*1024 + [0:1024],
                # 2048 + b*1024 + [0:1024]} -> out[2p+b, m-tile rows, 0:2048]
                # (contiguous 2KB DRAM lines)
                r = o01[:, :].rearrange("p (h b x) -> p h b x", h=2, b=2)
                for b in range(2):
                    dst = out[2 * p + b, 128 * t : 128 * (t + 1), :].rearrange(
                        "p (h x) -> p h x", h=2
                    )
                    getattr(nc, engines[b]).dma_start(out=dst, in_=r[:, :, b, :])

            for p in range(BPC // 2):       # batch pair
                for t in range(MT):
                    early = p == 0 and t < 3
                    last = p == BPC // 2 - 1 and t == MT - 1
                    o01 = out_pool.tile([128, 4096], i8, tag="o", name=f"o_{p}_{t}")
                    psA = psum_pool.tile([128, 1536], f32, tag="s3", name=f"A_{p}_{t}")
                    psB = psum_pool.tile([128, 1536], f32, tag="s3", name=f"B_{p}_{t}")
                    psC = psum_pool.tile([128, 1024], f32, tag="s2", bufs=1, name=f"C_{p}_{t}")
                    if early:
                        # bank-granular drains ordered by piece arrival:
                        # q0-gated banks first, then q1, then bh1
                        mm(psA, 0, p, t, 0)                  # b0 n0:512   (q0)
                        drain(o01, slice(0, 512), psA[:, 0:512], 512)
                        mm(psA, 2, p, t, 2)                  # b1 n0:512   (q0)
                        drain(o01, slice(1024, 1536), psA[:, 1024:1536], 512)
                        mm(psA, 1, p, t, 1)                  # b0 n512:1024 (q1)
                        drain(o01, slice(512, 1024), psA[:, 512:1024], 512)
                        mm(psB, 0, p, t, 3)                  # b1 n512:1024 (q1)
                        drain(o01, slice(1536, 2048), psB[:, 0:512], 512)
                        mm(psB, 1, p, t, 4)                  # b0 n1024:1536 (bh1)
                        mm(psB, 2, p, t, 5)                  # b0 n1536:2048 (bh1)
                        drain(o01, slice(2048, 3072), psB[:, 512:1536], 1024)
                        mm(psC, 0, p, t, 6)
                        mm(psC, 1, p, t, 7)
                        drain(o01, slice(3072, 4096), psC[:, :], 1024)
                        tile_dmas(o01, p, t)
                    elif not last:
                        mm(psA, 0, p, t, 0)
                        mm(psA, 1, p, t, 1)
                        mm(psA, 2, p, t, 2)
                        drain(o01, slice(0, 1536), psA[:, :], 1536)
                        mm(psB, 0, p, t, 3)
                        mm(psB, 1, p, t, 4)
                        mm(psB, 2, p, t, 5)
                        drain(o01, slice(1536, 3072), psB[:, :], 1536)
                        mm(psC, 0, p, t, 6)
                        mm(psC, 1, p, t, 7)
                        drain(o01, slice(3072, 4096), psC[:, :], 1024)
                        tile_dmas(o01, p, t)
                    else:
                        # final m-tile: sub-stripe drains with each 128KB
                        # quarter DMA'd the moment it completes, spread over
                        # sync / gpsimd / scalar; the last two drains are
                        # pinned (o0 h1 -> DVE with the sync trigger, o1 h1
                        # -> ACT with the scalar trigger) so the very last
                        # trigger starts the moment its own engine finishes.
                        mr = slice(128 * t, 128 * (t + 1))
                        mm(psA, 0, p, t, 0)
                        mm(psA, 1, p, t, 1)
                        drain(o01, slice(0, 1024), psA[:, 0:1024], 1024)
                        nc.sync.dma_start(
                            out=out[2 * p, mr, 0:1024], in_=o01[:, 0:1024]
                        )
                        mm(psA, 2, p, t, 2)
                        drain(o01, slice(1024, 1536), psA[:, 1024:1536], 512)
                        mm(psB, 0, p, t, 3)
                        drain(o01, slice(1536, 2048), psB[:, 0:512], 512)
                        nc.gpsimd.dma_start(
                            out=out[2 * p + 1, mr, 0:1024], in_=o01[:, 1024:2048]
                        )
                        mm(psB, 1, p, t, 4)
                        mm(psB, 2, p, t, 5)
                        drain(o01, slice(2048, 3072), psB[:, 512:1536], 1024, force="dve")
                        nc.sync.dma_start(
                            out=out[2 * p, mr, 1024:2048], in_=o01[:, 2048:3072]
                        )
                        mm(psC, 0, p, t, 6)
                        mm(psC, 1, p, t, 7)
                        drain(o01, slice(3072, 4096), psC[:, :], 1024, force="act")
                        nc.scalar.dma_start(
                            out=out[2 * p + 1, mr, 1024:2048], in_=o01[:, 3072:4096]
                        )
    nc.compile()
    return nc


def prep_in_maps(a: np.ndarray, b: np.ndarray):
    """Per-core packed input tensors: [K-pair, seq] bf16 pieces concatenated
    in on-device streaming order (see _build docstring)."""
    aT = np.ascontiguousarray(a.transpose(0, 2, 1)).astype(ml_dtypes.bfloat16)
    bT = np.ascontiguousarray(b.transpose(0, 2, 1)).astype(ml_dtypes.bfloat16)
    aT = aT.reshape(NCORES, BPC // 2, 128, M)
    bT = bT.reshape(NCORES, BPC // 2, 128, N)
    W = np.concatenate(
        [
            aT[:, 0, :, 0:128],
            bT[:, 0, :, 0:1024],
            aT[:, 0, :, 128:384],
            bT[:, 0, :, 1024:2048],
            aT[:, 0, :, 384:1024],
            aT[:, 0, :, 1024:2048],
            bT[:, 1],
            aT[:, 1],
        ],
        axis=2,
    )
    assert W.shape == (NCORES, 128, WCOLS)
    return [{"w": np.ascontiguousarray(W[c])} for c in range(NCORES)]


def kernel(a: np.ndarray, b: np.ndarray, alpha) -> np.ndarray:
    from concourse.bass_utils import run_bass_kernel_spmd

    a = np.asarray(a)
    b = np.asarray(b)
    alpha_f = float(np.asarray(alpha))

    key = alpha_f
    if key not in _CACHE:
        _CACHE[key] = _build(alpha_f)
    nc = _CACHE[key]

    in_maps = prep_in_maps(a, b)
    try:
        res = run_bass_kernel_spmd(nc, in_maps, core_ids=list(range(NCORES)))
    except Exception:
        # one retry in case a previous process left a device in a bad state
        res = run_bass_kernel_spmd(nc, in_maps, core_ids=list(range(NCORES)))
    outs = [res.results[c]["out"] for c in range(NCORES)]
    return np.concatenate(outs, axis=0).astype(np.int8)


# revision 13
# speedup vs baseline: 1.3140x; 1.3140x over previous
"""Bass/TRN2 kernel for nn_BMM_S8T_S8N_S8T:
    out[b,m,n] = sat_i8(round(alpha * sum_k a[b,m,k] * b[b,n,k]))
with a: (32, 2048, 64) int8, b: (32, 2048, 64) int8, alpha: f32 scalar.

Sharding: batch dim 32 -> 8 cores x 4 batches (pure data parallel, no
cross-core communication).

Per-core design notes:
  - int8 matmul is not supported by the PE; bf16 x bf16 -> f32 PSUM is exact
    for int8 operands (products < 2^16, sums of 64 < 2^24), so inputs are
    converted to bf16 (and pre-transposed to [K, seq] layout) on host.
  - The 4 local batches are stacked in pairs along SBUF partitions:
    partitions 0-63 hold batch 2p's K=64, partitions 64-127 batch 2p+1's.
    Row-tiled matmuls (row groups 0 and 2) run the two batches' K=64
    contractions on the 128x128 PE array.
  - Requant drain (PSUM f32 -> SBUF int8, scale + round-half-even + saturate)
    is the hard floor: only VectorE (0.96 GHz, 1 elem/lane/cyc from f32 PSUM)
    and ScalarE (1.2 GHz, 1 elem/lane/cyc) can read PSUM. Both engines drain
    in parallel on different PSUM banks, in [128, 1024] (2-bank) units so
    fills overlap drains within the 8 banks. A single tensor_scalar_mul /
    activation(Copy, scale) does the whole requant bit-exactly.
  - The 16 DMA engines are per-descriptor bound and serve descriptors in
    global trigger order, so all per-core input data is host-repacked into
    ONE dram tensor `w` laid out in exact streaming order and loaded as 5
    sequential pieces on the sync HWDGE; the first piece alone carries
    everything the first m-tile needs. Output DMAs ride the sync HWDGE too
    (a dedicated queue whose cross-engine waits stall nothing); the final
    m-tile's four [128,1024] halves are DMA'd individually from sync /
    gpsimd / scalar with the last two drains pinned to a known engine, so
    the exit tail is a single 128KB transfer.
"""

import numpy as np
import ml_dtypes

B, M, N, K = 32, 2048, 2048, 64
NCORES = 8
BPC = B // NCORES          # batches per core (4)
MT = M // 128              # m-tiles per batch (16)
NHALF = 2                  # two 1024-col drain units per m-tile row block
UNIT = N // NHALF          # 1024 columns per drain unit
WCOLS = 4 * M              # packed input tensor: 8192 bf16 columns

_CACHE = {}


def _build(alpha: float):
    import concourse.bacc as bacc
    import concourse.mybir as mybir
    from concourse.tile import TileContext

    bf16 = mybir.dt.bfloat16
    f32 = mybir.dt.float32
    i8 = mybir.dt.int8

    nc = bacc.Bacc("TRN2")
    w = nc.dram_tensor("w", (128, WCOLS), bf16, kind="ExternalInput")
    out = nc.dram_tensor("out", (BPC, M, N), i8, kind="ExternalOutput")

    # engine load balancing between the two drain engines (ns per [128,1024]
    # unit, hardware-measured)
    DVE_NS, ACT_NS = 1146.0, 1035.0
    WARMUP_MM = 4  # filler matmuls while the first input pieces stream in

    # scratch operand for warm-up matmuls: deliberately never written (the
    # values don't matter and the scratch PSUM bank is never read); a raw
    # (non-pool) tensor so Tile's release tracking doesn't object.
    wz = nc.alloc_sbuf_tensor("wz", [128, 512], bf16)

    with TileContext(nc) as tc:
        with (
            tc.tile_pool(name="inp", bufs=1) as inp_pool,
            tc.tile_pool(name="ps", bufs=4, space="PSUM") as psum_pool,
            tc.tile_pool(name="outp", bufs=10) as out_pool,
        ):
            # Input pieces, in streaming order (= DRAM column order of `w`).
            # The first m-tile's lhs (a0 cols 0:128) loads on the scalar
            # HWDGE in parallel with the first b piece's trigger generation
            # on the sync HWDGE; everything else follows on sync in order of
            # first use — the 16 DMA engines serve descriptors in global
            # trigger order, so this order IS the completion order. The b
            # quarters stay as two separate 512-col pieces (the first
            # matmuls start on the first piece's completion sem alone —
            # merging them measures slower end-to-end), and the tiny wa1
            # piece (a0 cols 128:384, 64KB) lands right behind them to
            # unblock m-tiles 1-2 (which reuse those b columns), bridging
            # the drain engines across the window where the bigger pieces
            # are still streaming.
            #   f0:  a0 cols 0:128     (scalar HWDGE)  -> m-tile 0 lhs
            #   q0:  b0 cols 0:512, q1: b0 cols 512:1024 -> h=0 rhs
            #   wa1: a0 cols 128:384                   -> m-tiles 1-2 lhs
            #   bh1: b0 cols 1024:2048                 -> h=1 rhs
            #   wa2: a0 cols 384:1024                  -> m-tiles 3-7 lhs
            #   wa3: a0 cols 1024:2048                 -> m-tiles 8-15 lhs
            #   tw3: b1 cols 0:2048, tw4: a1 cols 0:2048 -> batch pair 1
            f0 = inp_pool.tile([128, 128], bf16, tag="f0")
            q0 = inp_pool.tile([128, 512], bf16, tag="q0")
            q1 = inp_pool.tile([128, 512], bf16, tag="q1")
            wa1 = inp_pool.tile([128, 256], bf16, tag="wa1")
            bh1 = inp_pool.tile([128, 1024], bf16, tag="bh1")
            wa2 = inp_pool.tile([128, 640], bf16, tag="wa2")
            wa3 = inp_pool.tile([128, 1024], bf16, tag="wa3")
            tw3 = inp_pool.tile([128, 2048], bf16, tag="tw3")
            tw4 = inp_pool.tile([128, 2048], bf16, tag="tw4")
            # wa1 is triggered BEFORE q1: with m-tiles 0-1's h=0 units
            # drained at bank (512-col) granularity, q0+wa1 supply four
            # drainable banks ~1.2us before q1 lands, letting the drain
            # engines start gapless at the supply bound (~10.4us) instead
            # of waiting for q1 (~11.6us).
            nc.scalar.dma_start(out=f0[:, :], in_=w[:, 0:128])
            nc.sync.dma_start(out=q0[:, :], in_=w[:, 128:640])
            nc.sync.dma_start(out=wa1[:, :], in_=w[:, 1152:1408])
            nc.sync.dma_start(out=q1[:, :], in_=w[:, 640:1152])
            nc.sync.dma_start(out=bh1[:, :], in_=w[:, 1408:2432])
            nc.sync.dma_start(out=wa2[:, :], in_=w[:, 2432:3072])
            nc.sync.dma_start(out=wa3[:, :], in_=w[:, 3072:4096])
            nc.sync.dma_start(out=tw3[:, :], in_=w[:, 4096:6144])
            nc.sync.dma_start(out=tw4[:, :], in_=w[:, 6144:8192])

            # warm-up matmuls: keep the PE active (HAM clock ramp) while the
            # first input piece lands
            wps = psum_pool.tile([128, UNIT], f32, tag="ps")
            for _ in range(WARMUP_MM):
                nc.tensor.matmul(
                    wps[:, 0:512], wz[:, 0:128], wz[:, 0:512], start=True, stop=True
                )

            def lhs_ap(p, t, rows):
                if p == 1:
                    return tw4[rows, 128 * t : 128 * (t + 1)]
                if t == 0:
                    return f0[rows, :]
                if t < 3:
                    return wa1[rows, 128 * (t - 1) : 128 * t]
                if t < 8:
                    return wa2[rows, 128 * (t - 3) : 128 * (t - 2)]
                return wa3[rows, 128 * (t - 8) : 128 * (t - 7)]

            def rhs_ap(p, n0, rows):
                if p == 1:
                    return tw3[rows, n0 : n0 + 512]
                if n0 < 512:
                    return q0[rows, :]
                if n0 < 1024:
                    return q1[rows, :]
                return bh1[rows, n0 - 1024 : n0 - 512]

            dve_t = act_t = 0.0
            DVE_NS5, ACT_NS5 = 612.0, 609.0

            def drain(o, hs, ps_ap, dve_ns, act_ns, force=None):
                nonlocal dve_t, act_t
                use_dve = dve_t + dve_ns <= act_t + act_ns
                if force is not None:
                    use_dve = force == "dve"
                if use_dve:
                    nc.vector.tensor_scalar_mul(o[:, hs], ps_ap, alpha)
                    dve_t += dve_ns
                else:
                    nc.scalar.activation(
                        o[:, hs],
                        ps_ap,
                        mybir.ActivationFunctionType.Copy,
                        scale=alpha,
                    )
                    act_t += act_ns

            # ---- startup prelude: m-tiles 0-1, h=0, drained at bank (512
            # col) granularity in piece-arrival order — all four q0-gated
            # banks (both batches of t=0,1) before the four q1-gated banks.
            # This keeps both drain engines gapless from the moment q0+f0
            # land (~10.4us) instead of idling until q1 (~11.6us). PE writes
            # and engine reads stay on different banks of each pool tile.
            pre = {}
            for t in (0, 1):
                o0 = out_pool.tile([128, N], i8, tag="o", name=f"o0p_{t}")
                o1 = out_pool.tile([128, N], i8, tag="o", name=f"o1p_{t}")
                ps0 = psum_pool.tile([128, UNIT], f32, tag="ps", name=f"pre0_{t}")
                ps1 = psum_pool.tile([128, UNIT], f32, tag="ps", name=f"pre1_{t}")
                pre[t] = (ps0, ps1, o0, o1)
            for j in range(2):              # j=0 banks (q0), then j=1 (q1)
                c = slice(512 * j, 512 * (j + 1))
                for t in (0, 1):
                    ps0, ps1, o0, o1 = pre[t]
                    nc.tensor.matmul(
                        ps0[:, c],
                        lhs_ap(0, t, slice(0, 64)),
                        rhs_ap(0, 512 * j, slice(0, 64)),
                        start=True,
                        stop=True,
                    )
                    drain(o0, c, ps0[:, c], DVE_NS5, ACT_NS5)
                    nc.tensor.matmul(
                        ps1[:, c],
                        lhs_ap(0, t, slice(64, 128)),
                        rhs_ap(0, 512 * j, slice(64, 128)),
                        start=True,
                        stop=True,
                    )
                    drain(o1, c, ps1[:, c], DVE_NS5, ACT_NS5)

            for p in range(BPC // 2):       # batch pair
                # For pair 0, emit the remaining first three m-tiles' h=0
                # units before their h=1 units: the high b columns (in bh1)
                # land a bit after the low ones, and in-order emission would
                # stall the drain engines on t0's h=1 while t2's h=0 data is
                # ready.
                if p == 0:
                    order = [(2, 0), (0, 1), (1, 1), (2, 1)]
                    order += [(t, h) for t in range(3, MT) for h in range(NHALF)]
                    otiles = {0: pre[0][2:], 1: pre[1][2:]}
                    hdone = {0: 1, 1: 1}
                else:
                    order = [(t, h) for t in range(MT) for h in range(NHALF)]
                    otiles, hdone = {}, {}
                for t, h in order:
                    lhs0 = lhs_ap(p, t, slice(0, 64))
                    lhs1 = lhs_ap(p, t, slice(64, 128))
                    if t not in otiles:
                        o0 = out_pool.tile([128, N], i8, tag="o", name=f"o0_{p}_{t}")
                        o1 = out_pool.tile([128, N], i8, tag="o", name=f"o1_{p}_{t}")
                        otiles[t] = (o0, o1)
                        hdone[t] = 0
                    o0, o1 = otiles[t]
                    last = p == BPC // 2 - 1 and t == MT - 1
                    ps0 = psum_pool.tile([128, UNIT], f32, tag="ps", name=f"ps0_{p}_{t}_{h}")
                    ps1 = psum_pool.tile([128, UNIT], f32, tag="ps", name=f"ps1_{p}_{t}_{h}")
                    for j in range(2):  # 512-col matmul within unit
                        n0 = UNIT * h + 512 * j
                        c = slice(512 * j, 512 * (j + 1))
                        nc.tensor.matmul(
                            ps0[:, c],
                            lhs0,
                            rhs_ap(p, n0, slice(0, 64)),
                            start=True,
                            stop=True,
                        )
                        nc.tensor.matmul(
                            ps1[:, c],
                            lhs1,
                            rhs_ap(p, n0, slice(64, 128)),
                            start=True,
                            stop=True,
                        )
                    hs = slice(UNIT * h, UNIT * (h + 1))
                    tail = p == BPC // 2 - 1 and t == MT - 2
                    if not last:
                        drain(o0, hs, ps0[:, :], DVE_NS, ACT_NS)
                        if tail:
                            # second-to-last m-tile: fire each 128KB half as
                            # it drains so the sync queue's residual backlog
                            # at the final drain is one piece, not 512KB
                            nc.sync.dma_start(
                                out=out[2 * p, 128 * t : 128 * (t + 1), hs],
                                in_=o0[:, hs],
                            )
                        drain(o1, hs, ps1[:, :], DVE_NS, ACT_NS)
                        if tail:
                            nc.sync.dma_start(
                                out=out[2 * p + 1, 128 * t : 128 * (t + 1), hs],
                                in_=o1[:, hs],
                            )
                    else:
                        # final m-tile: per-half 128KB output DMAs fired as
                        # soon as each half is drained, spread over sync /
                        # gpsimd / scalar trigger queues; the h=1 drains are
                        # pinned (o0->DVE with the sync trigger, o1->ACT with
                        # the scalar trigger) so the very last DMA trigger
                        # starts the moment its own engine finishes and waits
                        # on nothing else.
                        force0 = force1 = None
                        if h == 1:
                            force0, force1 = "dve", "act"
                        drain(o0, hs, ps0[:, :], DVE_NS, ACT_NS, force=force0)
                        nc.sync.dma_start(
                            out=out[2 * p, 128 * t : 128 * (t + 1), hs],
                            in_=o0[:, hs],
                        )
                        drain(o1, hs, ps1[:, :], DVE_NS, ACT_NS, force=force1)
                        if h == 0:
                            nc.gpsimd.dma_start(
                                out=out[2 * p + 1, 128 * t : 128 * (t + 1), hs],
                                in_=o1[:, hs],
                            )
                        else:
                            nc.scalar.dma_start(
                                out=out[2 * p + 1, 128 * t : 128 * (t + 1), hs],
                                in_=o1[:, hs],
                            )
                    hdone[t] += 1
                    if hdone[t] < NHALF or last or tail:
                        continue
                    # full-tile output DMAs on the sync HWDGE ring (a
                    # dedicated queue that can afford to block on the
                    # cross-engine drain-completion waits)
                    nc.sync.dma_start(
                        out=out[2 * p, 128 * t : 128 * (t + 1), :], in_=o0[:, :]
                    )
                    nc.sync.dma_start(
                        out=out[2 * p + 1, 128 * t : 128 * (t + 1), :], in_=o1[:, :]
                    )
    nc.compile()
    return nc


def prep_in_maps(a: np.ndarray, b: np.ndarray):
    """Per-core packed input tensors: [K-pair, seq] bf16 pieces concatenated
    in on-device streaming order (see _build docstring)."""
    aT = np.ascontiguousarray(a.transpose(0, 2, 1)).astype(ml_dtypes.bfloat16)
    bT = np.ascontiguousarray(b.transpose(0, 2, 1)).astype(ml_dtypes.bfloat16)
    aT = aT.reshape(NCORES, BPC // 2, 128, M)
    bT = bT.reshape(NCORES, BPC // 2, 128, N)
    W = np.concatenate(
        [
            aT[:, 0, :, 0:128],
            bT[:, 0, :, 0:1024],
            aT[:, 0, :, 128:384],
            bT[:, 0, :, 1024:2048],
            aT[:, 0, :, 384:1024],
            aT[:, 0, :, 1024:2048],
            bT[:, 1],
            aT[:, 1],
        ],
        axis=2,
    )
    assert W.shape == (NCORES, 128, WCOLS)
    return [{"w": np.ascontiguousarray(W[c])} for c in range(NCORES)]


def kernel(a: np.ndarray, b: np.ndarray, alpha) -> np.ndarray:
    from concourse.bass_utils import run_bass_kernel_spmd

    a = np.asarray(a)
    b = np.asarray(b)
    alpha_f = float(np.asarray(alpha))

    key = alpha_f
    if key not in _CACHE:
        _CACHE[key] = _build(alpha_f)
    nc = _CACHE[key]

    in_maps = prep_in_maps(a, b)
    try:
        res = run_bass_kernel_spmd(nc, in_maps, core_ids=list(range(NCORES)))
    except Exception:
        # one retry in case a previous process left a device in a bad state
        res = run_bass_kernel_spmd(nc, in_maps, core_ids=list(range(NCORES)))
    outs = [res.results[c]["out"] for c in range(NCORES)]
    return np.concatenate(outs, axis=0).astype(np.int8)



# revision 14
# speedup vs baseline: 1.3192x; 1.0039x over previous
"""Bass/TRN2 kernel for nn_BMM_S8T_S8N_S8T:
    out[b,m,n] = sat_i8(round(alpha * sum_k a[b,m,k] * b[b,n,k]))
with a: (32, 2048, 64) int8, b: (32, 2048, 64) int8, alpha: f32 scalar.

Sharding: batch dim 32 -> 8 cores x 4 batches (pure data parallel, no
cross-core communication).

Per-core design notes:
  - int8 matmul is not supported by the PE; bf16 x bf16 -> f32 PSUM is exact
    for int8 operands (products < 2^16, sums of 64 < 2^24), so inputs are
    converted to bf16 (and pre-transposed to [K, seq] layout) on host.
  - The 4 local batches are stacked in pairs along SBUF partitions:
    partitions 0-63 hold batch 2p's K=64, partitions 64-127 batch 2p+1's.
    Row-tiled matmuls (row groups 0 and 2) run the two batches' K=64
    contractions on the 128x128 PE array.
  - Requant drain (PSUM f32 -> SBUF int8, scale + round-half-even + saturate)
    is the hard floor: only VectorE (0.96 GHz, 1 elem/lane/cyc from f32 PSUM)
    and ScalarE (1.2 GHz, 1 elem/lane/cyc) can read PSUM. Both engines drain
    in parallel on different PSUM banks, in [128, 1024] (2-bank) units so
    fills overlap drains within the 8 banks. A single tensor_scalar_mul /
    activation(Copy, scale) does the whole requant bit-exactly.
  - The 16 DMA engines are per-descriptor bound and serve descriptors in
    global trigger order, so all per-core input data is host-repacked into
    ONE dram tensor `w` laid out in exact streaming order and loaded as 5
    sequential pieces on the sync HWDGE; the first piece alone carries
    everything the first m-tile needs. Output DMAs ride the sync HWDGE too
    (a dedicated queue whose cross-engine waits stall nothing); the final
    m-tile's four [128,1024] halves are DMA'd individually from sync /
    gpsimd / scalar with the last two drains pinned to a known engine, so
    the exit tail is a single 128KB transfer.
"""

import numpy as np
import ml_dtypes

B, M, N, K = 32, 2048, 2048, 64
NCORES = 8
BPC = B // NCORES          # batches per core (4)
MT = M // 128              # m-tiles per batch (16)
NHALF = 2                  # two 1024-col drain units per m-tile row block
UNIT = N // NHALF          # 1024 columns per drain unit
WCOLS = 4 * M              # packed input tensor: 8192 bf16 columns

_CACHE = {}


def _build(alpha: float):
    import concourse.bacc as bacc
    import concourse.mybir as mybir
    from concourse.tile import TileContext

    bf16 = mybir.dt.bfloat16
    f32 = mybir.dt.float32
    i8 = mybir.dt.int8

    nc = bacc.Bacc("TRN2")
    w = nc.dram_tensor("w", (128, WCOLS), bf16, kind="ExternalInput")
    out = nc.dram_tensor("out", (BPC, M, N), i8, kind="ExternalOutput")

    # engine load balancing between the two drain engines (ns per [128,1024]
    # unit, hardware-measured)
    DVE_NS, ACT_NS = 1146.0, 1035.0
    WARMUP_MM = 4  # filler matmuls while the first input pieces stream in

    # scratch operand for warm-up matmuls: deliberately never written (the
    # values don't matter and the scratch PSUM bank is never read); a raw
    # (non-pool) tensor so Tile's release tracking doesn't object.
    wz = nc.alloc_sbuf_tensor("wz", [128, 512], bf16)

    with TileContext(nc) as tc:
        with (
            tc.tile_pool(name="inp", bufs=1) as inp_pool,
            tc.tile_pool(name="ps", bufs=4, space="PSUM") as psum_pool,
            tc.tile_pool(name="outp", bufs=10) as out_pool,
        ):
            # Input pieces, in streaming order (= DRAM column order of `w`).
            # The first m-tile's lhs (a0 cols 0:128) loads on the scalar
            # HWDGE in parallel with the first b piece's trigger generation
            # on the sync HWDGE; everything else follows on sync in order of
            # first use — the 16 DMA engines serve descriptors in global
            # trigger order, so this order IS the completion order. The b
            # quarters stay as two separate 512-col pieces (the first
            # matmuls start on the first piece's completion sem alone —
            # merging them measures slower end-to-end), and the tiny wa1
            # piece (a0 cols 128:384, 64KB) lands right behind them to
            # unblock m-tiles 1-2 (which reuse those b columns), bridging
            # the drain engines across the window where the bigger pieces
            # are still streaming.
            #   f0:  a0 cols 0:128     (scalar HWDGE)  -> m-tile 0 lhs
            #   q0:  b0 cols 0:512, q1: b0 cols 512:1024 -> h=0 rhs
            #   wa1: a0 cols 128:384                   -> m-tiles 1-2 lhs
            #   bh1: b0 cols 1024:2048                 -> h=1 rhs
            #   wa2: a0 cols 384:1024                  -> m-tiles 3-7 lhs
            #   wa3: a0 cols 1024:2048                 -> m-tiles 8-15 lhs
            #   tw3: b1 cols 0:2048, tw4: a1 cols 0:2048 -> batch pair 1
            f0 = inp_pool.tile([128, 128], bf16, tag="f0")
            q0 = inp_pool.tile([128, 512], bf16, tag="q0")
            q1 = inp_pool.tile([128, 512], bf16, tag="q1")
            wa1 = inp_pool.tile([128, 256], bf16, tag="wa1")
            bh1 = inp_pool.tile([128, 1024], bf16, tag="bh1")
            wa2 = inp_pool.tile([128, 640], bf16, tag="wa2")
            wa3 = inp_pool.tile([128, 1024], bf16, tag="wa3")
            tw3 = inp_pool.tile([128, 2048], bf16, tag="tw3")
            tw4 = inp_pool.tile([128, 2048], bf16, tag="tw4")
            nc.scalar.dma_start(out=f0[:, :], in_=w[:, 0:128])
            nc.sync.dma_start(out=q0[:, :], in_=w[:, 128:640])
            nc.sync.dma_start(out=q1[:, :], in_=w[:, 640:1152])
            nc.sync.dma_start(out=wa1[:, :], in_=w[:, 1152:1408])
            nc.sync.dma_start(out=bh1[:, :], in_=w[:, 1408:2432])
            nc.sync.dma_start(out=wa2[:, :], in_=w[:, 2432:3072])
            nc.sync.dma_start(out=wa3[:, :], in_=w[:, 3072:4096])
            nc.sync.dma_start(out=tw3[:, :], in_=w[:, 4096:6144])
            nc.sync.dma_start(out=tw4[:, :], in_=w[:, 6144:8192])

            # warm-up matmuls: keep the PE active (HAM clock ramp) while the
            # first input piece lands
            wps = psum_pool.tile([128, UNIT], f32, tag="ps")
            for _ in range(WARMUP_MM):
                nc.tensor.matmul(
                    wps[:, 0:512], wz[:, 0:128], wz[:, 0:512], start=True, stop=True
                )

            def lhs_ap(p, t, rows):
                if p == 1:
                    return tw4[rows, 128 * t : 128 * (t + 1)]
                if t == 0:
                    return f0[rows, :]
                if t < 3:
                    return wa1[rows, 128 * (t - 1) : 128 * t]
                if t < 8:
                    return wa2[rows, 128 * (t - 3) : 128 * (t - 2)]
                return wa3[rows, 128 * (t - 8) : 128 * (t - 7)]

            def rhs_ap(p, n0, rows):
                if p == 1:
                    return tw3[rows, n0 : n0 + 512]
                if n0 < 512:
                    return q0[rows, :]
                if n0 < 1024:
                    return q1[rows, :]
                return bh1[rows, n0 - 1024 : n0 - 512]

            dve_t = act_t = 0.0

            def drain(o, hs, ps_ap, dve_ns, act_ns, force=None):
                nonlocal dve_t, act_t
                use_dve = dve_t + dve_ns <= act_t + act_ns
                if force is not None:
                    use_dve = force == "dve"
                if use_dve:
                    nc.vector.tensor_scalar_mul(o[:, hs], ps_ap, alpha)
                    dve_t += dve_ns
                else:
                    nc.scalar.activation(
                        o[:, hs],
                        ps_ap,
                        mybir.ActivationFunctionType.Copy,
                        scale=alpha,
                    )
                    act_t += act_ns

            for p in range(BPC // 2):       # batch pair
                # For pair 0, emit the first three m-tiles' h=0 units before
                # their h=1 units: the high b columns (in tw1) land a bit
                # after the low ones, and in-order emission would stall the
                # drain engines on t0's h=1 while t1/t2's h=0 data is ready.
                if p == 0:
                    order = [(0, 0), (1, 0), (2, 0), (0, 1), (1, 1), (2, 1)]
                    order += [(t, h) for t in range(3, MT) for h in range(NHALF)]
                else:
                    order = [(t, h) for t in range(MT) for h in range(NHALF)]
                otiles, hdone = {}, {}
                for t, h in order:
                    lhs0 = lhs_ap(p, t, slice(0, 64))
                    lhs1 = lhs_ap(p, t, slice(64, 128))
                    if t not in otiles:
                        o0 = out_pool.tile([128, N], i8, tag="o", name=f"o0_{p}_{t}")
                        o1 = out_pool.tile([128, N], i8, tag="o", name=f"o1_{p}_{t}")
                        otiles[t] = (o0, o1)
                        hdone[t] = 0
                    o0, o1 = otiles[t]
                    last = p == BPC // 2 - 1 and t == MT - 1
                    ps0 = psum_pool.tile([128, UNIT], f32, tag="ps", name=f"ps0_{p}_{t}_{h}")
                    ps1 = psum_pool.tile([128, UNIT], f32, tag="ps", name=f"ps1_{p}_{t}_{h}")
                    for j in range(2):  # 512-col matmul within unit
                        n0 = UNIT * h + 512 * j
                        c = slice(512 * j, 512 * (j + 1))
                        nc.tensor.matmul(
                            ps0[:, c],
                            lhs0,
                            rhs_ap(p, n0, slice(0, 64)),
                            start=True,
                            stop=True,
                        )
                        nc.tensor.matmul(
                            ps1[:, c],
                            lhs1,
                            rhs_ap(p, n0, slice(64, 128)),
                            start=True,
                            stop=True,
                        )
                    hs = slice(UNIT * h, UNIT * (h + 1))
                    if not last:
                        drain(o0, hs, ps0[:, :], DVE_NS, ACT_NS)
                        drain(o1, hs, ps1[:, :], DVE_NS, ACT_NS)
                    else:
                        # final m-tile: per-half 128KB output DMAs fired as
                        # soon as each half is drained, spread over sync /
                        # gpsimd / scalar trigger queues; the h=1 drains are
                        # pinned (o0->DVE with the sync trigger, o1->ACT with
                        # the scalar trigger) so the very last DMA trigger
                        # starts the moment its own engine finishes and waits
                        # on nothing else.
                        force0 = force1 = None
                        if h == 1:
                            force0, force1 = "dve", "act"
                        drain(o0, hs, ps0[:, :], DVE_NS, ACT_NS, force=force0)
                        nc.sync.dma_start(
                            out=out[2 * p, 128 * t : 128 * (t + 1), hs],
                            in_=o0[:, hs],
                        )
                        drain(o1, hs, ps1[:, :], DVE_NS, ACT_NS, force=force1)
                        if h == 0:
                            nc.gpsimd.dma_start(
                                out=out[2 * p + 1, 128 * t : 128 * (t + 1), hs],
                                in_=o1[:, hs],
                            )
                        else:
                            nc.scalar.dma_start(
                                out=out[2 * p + 1, 128 * t : 128 * (t + 1), hs],
                                in_=o1[:, hs],
                            )
                    hdone[t] += 1
                    if hdone[t] < NHALF or last:
                        continue
                    # full-tile output DMAs on the sync HWDGE ring (a
                    # dedicated queue that can afford to block on the
                    # cross-engine drain-completion waits)
                    nc.sync.dma_start(
                        out=out[2 * p, 128 * t : 128 * (t + 1), :], in_=o0[:, :]
                    )
                    nc.sync.dma_start(
                        out=out[2 * p + 1, 128 * t : 128 * (t + 1), :], in_=o1[:, :]
                    )
    nc.compile()
    return nc


def prep_in_maps(a: np.ndarray, b: np.ndarray):
    """Per-core packed input tensors: [K-pair, seq] bf16 pieces concatenated
    in on-device streaming order (see _build docstring)."""
    aT = np.ascontiguousarray(a.transpose(0, 2, 1)).astype(ml_dtypes.bfloat16)
    bT = np.ascontiguousarray(b.transpose(0, 2, 1)).astype(ml_dtypes.bfloat16)
    aT = aT.reshape(NCORES, BPC // 2, 128, M)
    bT = bT.reshape(NCORES, BPC // 2, 128, N)
    W = np.concatenate(
        [
            aT[:, 0, :, 0:128],
            bT[:, 0, :, 0:1024],
            aT[:, 0, :, 128:384],
            bT[:, 0, :, 1024:2048],
            aT[:, 0, :, 384:1024],
            aT[:, 0, :, 1024:2048],
            bT[:, 1],
            aT[:, 1],
        ],
        axis=2,
    )
    assert W.shape == (NCORES, 128, WCOLS)
    return [{"w": np.ascontiguousarray(W[c])} for c in range(NCORES)]


def kernel(a: np.ndarray, b: np.ndarray, alpha) -> np.ndarray:
    from concourse.bass_utils import run_bass_kernel_spmd

    a = np.asarray(a)
    b = np.asarray(b)
    alpha_f = float(np.asarray(alpha))

    key = alpha_f
    if key not in _CACHE:
        _CACHE[key] = _build(alpha_f)
    nc = _CACHE[key]

    in_maps = prep_in_maps(a, b)
    try:
        res = run_bass_kernel_spmd(nc, in_maps, core_ids=list(range(NCORES)))
    except Exception:
        # one retry in case a previous process left a device in a bad state
        res = run_bass_kernel_spmd(nc, in_maps, core_ids=list(range(NCORES)))
    outs = [res.results[c]["out"] for c in range(NCORES)]
    return np.concatenate(outs, axis=0).astype(np.int8)



# revision 15
# speedup vs baseline: 1.3201x; 1.0007x over previous
"""Bass/TRN2 kernel for nn_BMM_S8T_S8N_S8T:
    out[b,m,n] = sat_i8(round(alpha * sum_k a[b,m,k] * b[b,n,k]))
with a: (32, 2048, 64) int8, b: (32, 2048, 64) int8, alpha: f32 scalar.

Sharding: batch dim 32 -> 8 cores x 4 batches (pure data parallel, no
cross-core communication).

Per-core design notes:
  - int8 matmul is not supported by the PE; bf16 x bf16 -> f32 PSUM is exact
    for int8 operands (products < 2^16, sums of 64 < 2^24), so inputs are
    converted to bf16 (and pre-transposed to [K, seq] layout) on host.
  - The 4 local batches are stacked in pairs along SBUF partitions:
    partitions 0-63 hold batch 2p's K=64, partitions 64-127 batch 2p+1's.
    Row-tiled matmuls (row groups 0 and 2) run the two batches' K=64
    contractions on the 128x128 PE array.
  - Requant drain (PSUM f32 -> SBUF int8, scale + round-half-even + saturate)
    is the hard floor: only VectorE (0.96 GHz, 1 elem/lane/cyc from f32 PSUM)
    and ScalarE (1.2 GHz, 1 elem/lane/cyc) can read PSUM. Both engines drain
    in parallel on different PSUM banks, in [128, 1024] (2-bank) units so
    fills overlap drains within the 8 banks. A single tensor_scalar_mul /
    activation(Copy, scale) does the whole requant bit-exactly.
  - The 16 DMA engines are per-descriptor bound and serve descriptors in
    global trigger order, so all per-core input data is host-repacked into
    ONE dram tensor `w` laid out in exact streaming order and loaded as 5
    sequential pieces on the sync HWDGE; the first piece alone carries
    everything the first m-tile needs. Output DMAs ride the sync HWDGE too
    (a dedicated queue whose cross-engine waits stall nothing); the final
    m-tile's four [128,1024] halves are DMA'd individually from sync /
    gpsimd / scalar with the last two drains pinned to a known engine, so
    the exit tail is a single 128KB transfer.
"""

import numpy as np
import ml_dtypes

B, M, N, K = 32, 2048, 2048, 64
NCORES = 8
BPC = B // NCORES          # batches per core (4)
MT = M // 128              # m-tiles per batch (16)
NHALF = 2                  # two 1024-col drain units per m-tile row block
UNIT = N // NHALF          # 1024 columns per drain unit
WCOLS = 4 * M              # packed input tensor: 8192 bf16 columns

_CACHE = {}


def _build(alpha: float):
    import concourse.bacc as bacc
    import concourse.mybir as mybir
    from concourse.tile import TileContext

    bf16 = mybir.dt.bfloat16
    f32 = mybir.dt.float32
    i8 = mybir.dt.int8

    nc = bacc.Bacc("TRN2")
    w = nc.dram_tensor("w", (128, WCOLS), bf16, kind="ExternalInput")
    out = nc.dram_tensor("out", (BPC, M, N), i8, kind="ExternalOutput")

    # engine load balancing between the two drain engines (ns per [128,1024]
    # unit, hardware-measured)
    DVE_NS, ACT_NS = 1146.0, 1035.0
    WARMUP_MM = 4  # filler matmuls while the first input pieces stream in

    # scratch operand for warm-up matmuls: deliberately never written (the
    # values don't matter and the scratch PSUM bank is never read); a raw
    # (non-pool) tensor so Tile's release tracking doesn't object.
    wz = nc.alloc_sbuf_tensor("wz", [128, 512], bf16)

    with TileContext(nc) as tc:
        with (
            tc.tile_pool(name="inp", bufs=1) as inp_pool,
            tc.tile_pool(name="ps", bufs=4, space="PSUM") as psum_pool,
            tc.tile_pool(name="outp", bufs=10) as out_pool,
        ):
            # Input pieces, in streaming order (= DRAM column order of `w`).
            # The first m-tile's lhs (a0 cols 0:128) loads on the scalar
            # HWDGE in parallel with the first b piece's trigger generation
            # on the sync HWDGE; everything else follows on sync in order of
            # first use — the 16 DMA engines serve descriptors in global
            # trigger order, so this order IS the completion order. The b
            # quarters stay as two separate 512-col pieces (the first
            # matmuls start on the first piece's completion sem alone —
            # merging them measures slower end-to-end), and the tiny wa1
            # piece (a0 cols 128:384, 64KB) lands right behind them to
            # unblock m-tiles 1-2 (which reuse those b columns), bridging
            # the drain engines across the window where the bigger pieces
            # are still streaming.
            #   f0:  a0 cols 0:128     (scalar HWDGE)  -> m-tile 0 lhs
            #   q0:  b0 cols 0:512, q1: b0 cols 512:1024 -> h=0 rhs
            #   wa1: a0 cols 128:384                   -> m-tiles 1-2 lhs
            #   bh1: b0 cols 1024:2048                 -> h=1 rhs
            #   wa2: a0 cols 384:1024                  -> m-tiles 3-7 lhs
            #   wa3: a0 cols 1024:2048                 -> m-tiles 8-15 lhs
            #   tw3: b1 cols 0:2048, tw4: a1 cols 0:2048 -> batch pair 1
            f0 = inp_pool.tile([128, 128], bf16, tag="f0")
            q0 = inp_pool.tile([128, 512], bf16, tag="q0")
            q1 = inp_pool.tile([128, 512], bf16, tag="q1")
            wa1 = inp_pool.tile([128, 256], bf16, tag="wa1")
            bh1 = inp_pool.tile([128, 1024], bf16, tag="bh1")
            wa23 = inp_pool.tile([128, 1664], bf16, tag="wa23")
            tw34 = inp_pool.tile([128, 4096], bf16, tag="tw34")
            nc.scalar.dma_start(out=f0[:, :], in_=w[:, 0:128])
            nc.sync.dma_start(out=q0[:, :], in_=w[:, 128:640])
            nc.sync.dma_start(out=q1[:, :], in_=w[:, 640:1152])
            nc.sync.dma_start(out=wa1[:, :], in_=w[:, 1152:1408])
            nc.sync.dma_start(out=bh1[:, :], in_=w[:, 1408:2432])
            nc.sync.dma_start(out=wa23[:, :], in_=w[:, 2432:4096])
            nc.sync.dma_start(out=tw34[:, :], in_=w[:, 4096:8192])

            # warm-up matmuls: keep the PE active (HAM clock ramp) while the
            # first input piece lands
            wps = psum_pool.tile([128, UNIT], f32, tag="ps")
            for _ in range(WARMUP_MM):
                nc.tensor.matmul(
                    wps[:, 0:512], wz[:, 0:128], wz[:, 0:512], start=True, stop=True
                )

            def lhs_ap(p, t, rows):
                if p == 1:
                    return tw34[rows, 2048 + 128 * t : 2048 + 128 * (t + 1)]
                if t == 0:
                    return f0[rows, :]
                if t < 3:
                    return wa1[rows, 128 * (t - 1) : 128 * t]
                return wa23[rows, 128 * (t - 3) : 128 * (t - 2)]

            def rhs_ap(p, n0, rows):
                if p == 1:
                    return tw34[rows, n0 : n0 + 512]
                if n0 < 512:
                    return q0[rows, :]
                if n0 < 1024:
                    return q1[rows, :]
                return bh1[rows, n0 - 1024 : n0 - 512]

            dve_t = act_t = 0.0

            def drain(o, hs, ps_ap, dve_ns, act_ns, force=None):
                nonlocal dve_t, act_t
                use_dve = dve_t + dve_ns <= act_t + act_ns
                if force is not None:
                    use_dve = force == "dve"
                if use_dve:
                    nc.vector.tensor_scalar_mul(o[:, hs], ps_ap, alpha)
                    dve_t += dve_ns
                else:
                    nc.scalar.activation(
                        o[:, hs],
                        ps_ap,
                        mybir.ActivationFunctionType.Copy,
                        scale=alpha,
                    )
                    act_t += act_ns

            for p in range(BPC // 2):       # batch pair
                # For pair 0, emit the first three m-tiles' h=0 units before
                # their h=1 units: the high b columns (in tw1) land a bit
                # after the low ones, and in-order emission would stall the
                # drain engines on t0's h=1 while t1/t2's h=0 data is ready.
                if p == 0:
                    order = [(0, 0), (1, 0), (2, 0), (0, 1), (1, 1), (2, 1)]
                    order += [(t, h) for t in range(3, MT) for h in range(NHALF)]
                else:
                    order = [(t, h) for t in range(MT) for h in range(NHALF)]
                otiles, hdone = {}, {}
                for t, h in order:
                    lhs0 = lhs_ap(p, t, slice(0, 64))
                    lhs1 = lhs_ap(p, t, slice(64, 128))
                    if t not in otiles:
                        o0 = out_pool.tile([128, N], i8, tag="o", name=f"o0_{p}_{t}")
                        o1 = out_pool.tile([128, N], i8, tag="o", name=f"o1_{p}_{t}")
                        otiles[t] = (o0, o1)
                        hdone[t] = 0
                    o0, o1 = otiles[t]
                    last = p == BPC // 2 - 1 and t == MT - 1
                    ps0 = psum_pool.tile([128, UNIT], f32, tag="ps", name=f"ps0_{p}_{t}_{h}")
                    ps1 = psum_pool.tile([128, UNIT], f32, tag="ps", name=f"ps1_{p}_{t}_{h}")
                    for j in range(2):  # 512-col matmul within unit
                        n0 = UNIT * h + 512 * j
                        c = slice(512 * j, 512 * (j + 1))
                        nc.tensor.matmul(
                            ps0[:, c],
                            lhs0,
                            rhs_ap(p, n0, slice(0, 64)),
                            start=True,
                            stop=True,
                        )
                        nc.tensor.matmul(
                            ps1[:, c],
                            lhs1,
                            rhs_ap(p, n0, slice(64, 128)),
                            start=True,
                            stop=True,
                        )
                    hs = slice(UNIT * h, UNIT * (h + 1))
                    tail = p == BPC // 2 - 1 and t == MT - 2
                    split = p == BPC // 2 - 1 and t == 12 and h == 0
                    if not last:
                        if split:
                            # one 768/256-granule unit lets the greedy
                            # balancer equalize the two engines' finish
                            # times below the 1024-unit quantization
                            drain(o0, slice(0, 768), ps0[:, 0:768], 879.0, 822.0)
                            drain(o0, slice(768, 1024), ps0[:, 768:1024], 346.0, 395.0)
                        else:
                            drain(o0, hs, ps0[:, :], DVE_NS, ACT_NS)
                        if tail:
                            # second-to-last m-tile: fire each 128KB half as
                            # it drains so the sync queue's residual backlog
                            # at the final drain is one piece, not 512KB
                            nc.sync.dma_start(
                                out=out[2 * p, 128 * t : 128 * (t + 1), hs],
                                in_=o0[:, hs],
                            )
                        drain(o1, hs, ps1[:, :], DVE_NS, ACT_NS)
                        if tail:
                            nc.sync.dma_start(
                                out=out[2 * p + 1, 128 * t : 128 * (t + 1), hs],
                                in_=o1[:, hs],
                            )
                    else:
                        # final m-tile: per-half 128KB output DMAs fired as
                        # soon as each half is drained, spread over sync /
                        # gpsimd / scalar trigger queues; the h=1 drains are
                        # pinned (o0->DVE with the sync trigger, o1->ACT with
                        # the scalar trigger) so the very last DMA trigger
                        # starts the moment its own engine finishes and waits
                        # on nothing else.
                        force0 = force1 = None
                        if h == 1:
                            force0, force1 = "dve", "act"
                        drain(o0, hs, ps0[:, :], DVE_NS, ACT_NS, force=force0)
                        nc.sync.dma_start(
                            out=out[2 * p, 128 * t : 128 * (t + 1), hs],
                            in_=o0[:, hs],
                        )
                        drain(o1, hs, ps1[:, :], DVE_NS, ACT_NS, force=force1)
                        if h == 0:
                            nc.gpsimd.dma_start(
                                out=out[2 * p + 1, 128 * t : 128 * (t + 1), hs],
                                in_=o1[:, hs],
                            )
                        else:
                            nc.scalar.dma_start(
                                out=out[2 * p + 1, 128 * t : 128 * (t + 1), hs],
                                in_=o1[:, hs],
                            )
                    hdone[t] += 1
                    if hdone[t] < NHALF or last or tail:
                        continue
                    # full-tile output DMAs on the sync HWDGE ring (a
                    # dedicated queue that can afford to block on the
                    # cross-engine drain-completion waits)
                    nc.sync.dma_start(
                        out=out[2 * p, 128 * t : 128 * (t + 1), :], in_=o0[:, :]
                    )
                    nc.sync.dma_start(
                        out=out[2 * p + 1, 128 * t : 128 * (t + 1), :], in_=o1[:, :]
                    )
    nc.compile()
    return nc


def prep_in_maps(a: np.ndarray, b: np.ndarray):
    """Per-core packed input tensors: [K-pair, seq] bf16 pieces concatenated
    in on-device streaming order (see _build docstring)."""
    aT = np.ascontiguousarray(a.transpose(0, 2, 1)).astype(ml_dtypes.bfloat16)
    bT = np.ascontiguousarray(b.transpose(0, 2, 1)).astype(ml_dtypes.bfloat16)
    aT = aT.reshape(NCORES, BPC // 2, 128, M)
    bT = bT.reshape(NCORES, BPC // 2, 128, N)
    W = np.concatenate(
        [
            aT[:, 0, :, 0:128],
            bT[:, 0, :, 0:1024],
            aT[:, 0, :, 128:384],
            bT[:, 0, :, 1024:2048],
            aT[:, 0, :, 384:1024],
            aT[:, 0, :, 1024:2048],
            bT[:, 1],
            aT[:, 1],
        ],
        axis=2,
    )
    assert W.shape == (NCORES, 128, WCOLS)
    return [{"w": np.ascontiguousarray(W[c])} for c in range(NCORES)]


def kernel(a: np.ndarray, b: np.ndarray, alpha) -> np.ndarray:
    from concourse.bass_utils import run_bass_kernel_spmd

    a = np.asarray(a)
    b = np.asarray(b)
    alpha_f = float(np.asarray(alpha))

    key = alpha_f
    if key not in _CACHE:
        _CACHE[key] = _build(alpha_f)
    nc = _CACHE[key]

    in_maps = prep_in_maps(a, b)
    try:
        res = run_bass_kernel_spmd(nc, in_maps, core_ids=list(range(NCORES)))
    except Exception:
        # one retry in case a previous process left a device in a bad state
        res = run_bass_kernel_spmd(nc, in_maps, core_ids=list(range(NCORES)))
    outs = [res.results[c]["out"] for c in range(NCORES)]
    return np.concatenate(outs, axis=0).astype(np.int8)

